# revision 43
# baseline (speedup 1.0000x reference)
"""Contrastive loss (soft-target NT-Xent) on 8 Trainium2 NeuronCores.

Math (matches the reference):
    e = x / max(||x||, eps)              row-normalized embeddings
    sim = e @ e.T / T                    T = 0.1
    logz_i = logsumexp_{j != i} sim[i, j]
    row_loss_i = sum_{j: l_j == l_i, j != i} (logz_i - sim[i, j])
    loss = sum_i row_loss_i / N

Decomposition: the device only computes the exp-sums; everything involving
labels / positive pairs / logs runs on the host.

Each unordered pair {i, j} is exp'd ONCE using a wrapped band: chunk c1
(128 rows) covers chunk-columns c1..c1+32 (mod 64).  Row-sums of an exp'd
block feed sumexp for its rows (free-axis accumulate); column-sums feed
sumexp for its columns (partition reduce via a ones^T @ exptile matmul
accumulated in PSUM).  The diagonal chunk (both orders present in the
block) and the antipodal chunk (distance 32, computed by both endpoints)
contribute row-sums only.  The band is shift invariant, so all 8 cores run
the identical program on inputs rotated by c*1024 samples; each core needs
only local columns [0, 5120).

Per core the device returns row-sum partials rowp [128, NSLOT] (one slot
per exp piece) and column-sum partials colp [1, 5120] (garbage outside
[128, 4992), ignored by the host).  Host: sumexp_i = rowparts + colparts -
exp(10*g_ii - 10) (exact diag from the quantized embeddings), logz = 10 +
ln(sumexp), row_loss = C_i*logz - 10*(d_i - 1) with d_i = e_i . S_{l_i}.

Exp pieces are split between ScalarE (exact exp, fused row accumulate) and
VectorE (Schraudolph float-bits int16 trick + 4x bf16 reduce), ratio
tunable via n_sc.
"""

import math

import numpy as np

import concourse.bass as bass
import concourse.bacc as bacc
import concourse.tile as tile
from concourse import mybir

N = 8192
D = 128
NCLASS = 100
NCORES = 8
CH = 64  # 128-row chunks
BAND = 33  # chunks per strip (diag + 32)
LCOLS = 7 * 128 + BAND * 128  # local column span = 5120
CSPAN_LO, CSPAN_HI = 128, 7 * 128 + 4096  # useful colsum range
TEMP_INV = 10.0

LOG2E = math.log2(math.e)
C16 = TEMP_INV * LOG2E * 2.0**16  # embedding scale^2 folded in on host
ACT_SCALE = math.log(2.0) / 2.0**16  # psum -> exp argument (with bias -10)
SCH_SIGMA = 0.0566  # Schraudolph mean-bias correction
SCH_B16 = (127.0 - SCH_SIGMA - TEMP_INV * LOG2E) * 2.0**7
SCH_K16 = 2.0**-9

F32 = mybir.dt.float32
BF16 = mybir.dt.bfloat16
I16 = mybir.dt.int16

SGW = 1024  # colacc super-group width (2 PSUM banks, double-buffered)
GROUPS = [(i * SGW, (i + 1) * SGW) for i in range(LCOLS // SGW)]
PIECE_W = 1024  # max exp piece width (psum tile, 2 banks)
COPY_ENG = ["S", "D", "S", "D", "S"]  # colacc->SBUF copy engine per group


def schedule(n_sc: int = 30):
    """Static per-core schedule (identical on all cores).

    Returns (groups, nslot): groups is a list of
    dict(G0, G1, batches=[dict(k, pieces=[(p0, p1, eng, slot)],
    cols=[(a, b, start, stop)])]).
    """
    slot = 0
    groups = []
    pieces_flat = []
    for G0, G1 in GROUPS:
        batches = []
        for k in range(8):
            cs, ce = max(k * 128, G0), min(k * 128 + BAND * 128, G1)
            if cs >= ce:
                continue
            pieces = []
            p = cs
            # tiny first piece so the exp pipeline starts right after the
            # first small DMA slice lands
            if cs == 0:
                pieces.append([0, 256, None, slot])
                slot += 1
                p = 256
            while p < ce:
                pe_ = min(p + PIECE_W, ce)
                pieces.append([p, pe_, None, slot])
                slot += 1
                p = pe_
            pieces_flat.extend(pieces)
            # colsum range: strip cols minus diag chunk minus antipode chunk
            c0, c1 = max(k * 128 + 128, cs), min(k * 128 + 4096, ce)
            cols = []
            if c0 < c1:
                bounds = {c0, c1}
                bounds.update(b for b in range(0, LCOLS + 1, 512) if c0 < b < c1)
                bounds.update(
                    pp for pc in pieces for pp in pc[:2] if c0 < pp < c1
                )
                bs = sorted(bounds)
                cols = [[a, b, False, False] for a, b in zip(bs[:-1], bs[1:])]
            batches.append(dict(k=k, cs=cs, ce=ce, pieces=pieces, cols=cols))
        groups.append(dict(G0=G0, G1=G1, batches=batches))

    # Engine assignment.  Diag chunks may land on either engine: the host
    # subtracts an engine-matched diag term (exact exp for ScalarE pieces,
    # a Schraudolph replica for VectorE pieces).
    if n_sc < 0:
        # width-aware greedy: assign each piece (in emission order) to the
        # engine that finishes it sooner, using per-instruction cost models
        # (ScalarE activation vs VectorE convert+reduce).  Copies pre-load
        # both engines.
        load_s = 5 * (172 + 512) / 1.2  # 5 colacc half-copies on ScalarE
        load_d = 5 * (120 + 512) / 0.96  # 5 on VectorE
        for pc in pieces_flat:
            w = pc[1] - pc[0]
            cs_ = (172 + w) / 1.2 + 70
            cd_ = (178 + 2 * w) / 0.96 + 100
            if load_s + cs_ <= load_d + cd_:
                pc[2] = "S"
                load_s += cs_
            else:
                pc[2] = "D"
                load_d += cd_
    else:
        total = len(pieces_flat)
        s_used = 0
        for i, pc in enumerate(pieces_flat):
            if s_used * total < n_sc * (i + 1) and s_used < n_sc:
                pc[2] = "S"
                s_used += 1
            else:
                pc[2] = "D"

    # start/stop flags per (group, psum bank of colacc)
    for g in groups:
        first_seen, last_mm = {}, {}
        for batch in g["batches"]:
            for cm in batch["cols"]:
                bank = (cm[0] - g["G0"]) // 512
                if bank not in first_seen:
                    cm[2] = True
                    first_seen[bank] = True
                last_mm[bank] = cm
        for cm in last_mm.values():
            cm[3] = True
    return groups, slot


def build_nc(
    loop_k: int = 1,
    n_sc: int = 30,
    stage: int = 3,
    preload: int = 0,
    warm: int = 4,
):
    groups, nslot = schedule(n_sc)
    nc = bacc.Bacc("TRN2", target_bir_lowering=False, debug=False)

    xt_d = nc.dram_tensor("xt", [128, LCOLS], BF16, kind="ExternalInput")
    rowp_d = nc.dram_tensor("rowp", [128, nslot], F32, kind="ExternalOutput")
    colp_d = nc.dram_tensor("colp", [1, LCOLS], F32, kind="ExternalOutput")

    with tile.TileContext(nc) as tc:
        with (
            tc.tile_pool(name="persist", bufs=1) as persist,
            tc.tile_pool(name="expool", bufs=6) as expool,
            tc.tile_pool(name="mpsum", bufs=3, space="PSUM") as mpsum,
            tc.tile_pool(name="colpsum", bufs=2, space="PSUM") as colpsum,
        ):
            xt = persist.tile([128, LCOLS], BF16, tag="xt")
            ones = persist.tile([128, 1], BF16, tag="ones")
            warmt = persist.tile([128, 512], BF16, tag="warmt")
            nc.vector.memset(warmt[:], 1.0)
            bneg10 = persist.tile([128, 1], F32, tag="bneg10")
            rowp = persist.tile([128, nslot], F32, tag="rowp")
            colsb = persist.tile([1, LCOLS], F32, tag="colsb")

            nc.vector.memset(ones[:], 1.0)
            nc.vector.memset(bneg10[:], -TEMP_INV)
            if stage < 2:
                nc.vector.memset(rowp[:], 0.0)

            def load_xt():
                for a, b in [(0, 256), (256, 1024)] + [
                    (i * 1024, (i + 1) * 1024) for i in range(1, 5)
                ]:
                    nc.sync.dma_start(xt[:, a:b], xt_d[:, a:b])

            def body():
                if not preload:
                    load_xt()
                if warm:
                    # dummy matmuls during the initial DMA wait keep the PE
                    # HAM activity window busy so real matmuls start warm
                    wps = colpsum.tile([128, 512], F32, tag="colacc", name="wps")
                    for _ in range(warm):
                        nc.tensor.matmul(
                            wps[:], warmt[:, 0:128], warmt[:], start=True,
                            stop=True, skip_group_check=True,
                        )

                for gi, g in enumerate(groups):
                    G0, G1 = g["G0"], g["G1"]
                    # two single-bank colacc halves (pool bufs=2 -> the next
                    # group's accumulation doesn't wait on this group's copy)
                    colacc_h = [
                        colpsum.tile(
                            [128, 512], F32, tag="colacc", name=f"colacc{h}"
                        )
                        for h in range((G1 - G0) // 512)
                    ]
                    ext_of = {}  # piece p0 -> (tile, p0, bf16_view)

                    def emit_colsums(batch):
                        for a, b, st, sp in batch["cols"]:
                            # find the piece containing [a, b)
                            for p0, p1, eng, slot_ in batch["pieces"]:
                                if p0 <= a and b <= p1:
                                    break
                            ext, q0, view = ext_of[p0]
                            h = (a - G0) // 512
                            nc.tensor.matmul(
                                colacc_h[h][0:1, a - G0 - 512 * h : b - G0 - 512 * h],
                                ones[:],
                                view[:, a - q0 : b - q0],
                                start=st,
                                stop=sp,
                                skip_group_check=True,
                            )

                    pending = None
                    for batch in g["batches"]:
                        k = batch["k"]
                        lhsT = xt[:, k * 128 : (k + 1) * 128]
                        for p0, p1, eng, slot_ in batch["pieces"]:
                            w = p1 - p0
                            ps = mpsum.tile([128, PIECE_W], F32, tag="ps")
                            for a in range(0, w, 512):
                                b = min(a + 512, w)
                                nc.tensor.matmul(
                                    ps[:, a:b],
                                    lhsT,
                                    xt[:, p0 + a : p0 + b],
                                    start=True,
                                    stop=True,
                                )
                            if stage < 2:
                                continue
                            if eng == "S":
                                ext = expool.tile(
                                    [128, PIECE_W], BF16, tag="extS", name="extS"
                                )
                                nc.scalar.activation(
                                    ext[:, :w],
                                    ps[:, :w],
                                    mybir.ActivationFunctionType.Exp,
                                    bias=bneg10[:],
                                    scale=ACT_SCALE,
                                    accum_out=rowp[:, slot_ : slot_ + 1],
                                )
                                ext_of[p0] = (ext, p0, ext)
                            else:
                                ext = expool.tile(
                                    [128, PIECE_W], I16, tag="extD", name="extD"
                                )
                                nc.vector.tensor_scalar(
                                    out=ext[:, :w],
                                    in0=ps[:, :w],
                                    scalar1=SCH_K16,
                                    scalar2=SCH_B16,
                                    op0=mybir.AluOpType.mult,
                                    op1=mybir.AluOpType.add,
                                )
                                nc.vector.tensor_reduce(
                                    out=rowp[:, slot_ : slot_ + 1],
                                    in_=ext[:, :w].bitcast(BF16),
                                    axis=mybir.AxisListType.X,
                                    op=mybir.AluOpType.add,
                                )
                                ext_of[p0] = (ext, p0, ext.bitcast(BF16))
                        if stage >= 3 and pending is not None:
                            emit_colsums(pending)
                        pending = batch
                    if stage >= 3:
                        emit_colsums(pending)
                        if gi == len(groups) - 1:
                            # rowp is complete once the last exps retire;
                            # start its DMA before the final copy+DMA tail
                            nc.sync.dma_start(rowp_d[:], rowp[:])
                        # copy halves on different engines (different banks)
                        nc.scalar.copy(
                            colsb[0:1, G0 : G0 + 512], colacc_h[0][0:1, :]
                        )
                        nc.vector.tensor_copy(
                            colsb[0:1, G0 + 512 : G1], colacc_h[1][0:1, :]
                        )
                        nc.sync.dma_start(colp_d[0:1, G0:G1], colsb[0:1, G0:G1])

                if stage < 3:
                    nc.sync.dma_start(rowp_d[:], rowp[:])

            if preload:
                load_xt()
            if loop_k == 1:
                body()
            else:
                with tc.For_i(0, loop_k, 1):
                    body()

    nc.compile()
    return nc


def prepare_inputs(embeddings: np.ndarray, labels: np.ndarray):
    """Host prep: normalize+scale rows to bf16; per-core rotated transposed
    views (only local columns [0, 5120)).  Returns (in_maps, aux) where aux
    carries everything the host combine step needs."""
    import ml_dtypes

    x = np.asarray(embeddings, dtype=np.float64)
    rn = 1.0 / np.maximum(np.sqrt((x * x).sum(axis=1)), 1e-12)
    e = x * rn[:, None]
    xn = (e * math.sqrt(C16)).astype(ml_dtypes.bfloat16)

    in_maps = []
    for c in range(NCORES):
        loc = np.roll(xn, -c * 1024, axis=0)
        xt_host = np.ascontiguousarray(loc.T[:, :LCOLS])
        in_maps.append({"xt": xt_host})

    lab = np.asarray(labels).astype(np.int64).ravel()
    xnf = xn.astype(np.float64)
    gdiag = (xnf * xnf).sum(axis=1) / C16
    diag_exact = np.exp(TEMP_INV * gdiag - TEMP_INV)
    # Schraudolph replica of the device's VectorE path for the diag term
    z = np.rint(gdiag * C16 * SCH_K16 + SCH_B16).astype(np.int16)
    diag_sch = z.view(np.uint16).astype(np.uint16).view(ml_dtypes.bfloat16)
    diag_sch = diag_sch.astype(np.float64)
    counts = np.bincount(lab, minlength=NCLASS).astype(np.float64)
    S = np.zeros((NCLASS, D))
    np.add.at(S, lab, e)
    d = np.einsum("ij,ij->i", e, S[lab])
    Ci = counts[lab] - 1.0
    aux = dict(diag_exact=diag_exact, diag_sch=diag_sch, d=d, Ci=Ci)
    return in_maps, aux


def combine(results, aux, n_sc: int = 30) -> np.ndarray:
    """Host combine: per-core rowp/colp partials -> loss."""
    groups, nslot = schedule(n_sc)
    # strip of each slot
    slot_strip = np.empty(nslot, dtype=np.int64)
    for g in groups:
        for batch in g["batches"]:
            for p0, p1, eng, slot_ in batch["pieces"]:
                slot_strip[slot_] = batch["k"]

    rowsum = np.zeros(N)
    colsum = np.zeros(N)
    ridx = np.arange(1024)
    cidx = np.arange(CSPAN_LO, CSPAN_HI)
    for c in range(NCORES):
        rowp = np.asarray(results[c]["rowp"], dtype=np.float64)  # [128, nslot]
        colp = np.asarray(results[c]["colp"], dtype=np.float64).ravel()
        lrow = np.zeros(1024)
        for k in range(8):
            sl = np.where(slot_strip == k)[0]
            lrow[k * 128 : (k + 1) * 128] = rowp[:, sl].sum(axis=1)
        rowsum[(ridx + c * 1024) % N] += lrow
        colsum[(cidx + c * 1024) % N] += colp[CSPAN_LO:CSPAN_HI]

    # per-row diag term: rows of local strip k use the engine of the piece
    # holding column k*128 in group 0 (same schedule on every core)
    diag_eng = {}
    for batch in groups[0]["batches"]:
        for p0, p1, eng, _s in batch["pieces"]:
            if p0 <= batch["k"] * 128 < p1:
                diag_eng[batch["k"]] = eng
    diag_term = aux["diag_exact"].copy()
    for c in range(NCORES):
        for k in range(8):
            if diag_eng[k] == "D":
                r0 = c * 1024 + k * 128
                diag_term[r0 : r0 + 128] = aux["diag_sch"][r0 : r0 + 128]

    sumexp = rowsum + colsum - diag_term
    logz = TEMP_INV + np.log(sumexp)
    row_loss = aux["Ci"] * logz - TEMP_INV * (aux["d"] - 1.0)
    return np.asarray(row_loss.sum() / N, dtype=np.float32)


_NC_CACHE = {}


def kernel(embeddings: np.ndarray, labels: np.ndarray) -> np.ndarray:
    from concourse.bass_utils import run_bass_kernel_spmd

    nc = _NC_CACHE.get("nc")
    if nc is None:
        nc = _NC_CACHE["nc"] = build_nc(loop_k=1)
    in_maps, aux = prepare_inputs(embeddings, labels)
    res = run_bass_kernel_spmd(nc, in_maps, list(range(NCORES)))
    return combine(res.results, aux)


# revision 44
# speedup vs baseline: 1.4443x; 1.4443x over previous
"""Contrastive loss (soft-target NT-Xent) on 8 Trainium2 NeuronCores.

Math (matches the reference):
    e = x / max(||x||, eps)              row-normalized embeddings
    sim = e @ e.T / T                    T = 0.1
    logz_i = logsumexp_{j != i} sim[i, j]
    row_loss_i = sum_{j: l_j == l_i, j != i} (logz_i - sim[i, j])
    loss = sum_i row_loss_i / N

Decomposition: the device only computes the exp-sums; everything involving
labels / positive pairs / logs runs on the host.

Each unordered pair {i, j} is exp'd ONCE using a wrapped band: chunk c1
(128 rows) covers chunk-columns c1..c1+32 (mod 64).  Row-sums of an exp'd
block feed sumexp for its rows (free-axis accumulate); column-sums feed
sumexp for its columns (partition reduce via a ones^T @ exptile matmul
accumulated in PSUM).  The diagonal chunk (both orders present in the
block) and the antipodal chunk (distance 32, computed by both endpoints)
contribute row-sums only.  The band is shift invariant, so all 8 cores run
the identical program on inputs rotated by c*1024 samples; each core needs
only local columns [0, 5120).

Per core the device returns row-sum partials rowp [128, NSLOT] (one slot
per exp piece) and column-sum partials colp [1, 5120] (garbage outside
[128, 4992), ignored by the host).  Host: sumexp_i = rowparts + colparts -
exp(10*g_ii - 10) (exact diag from the quantized embeddings), logz = 10 +
ln(sumexp), row_loss = C_i*logz - 10*(d_i - 1) with d_i = e_i . S_{l_i}.

Exp pieces are split between ScalarE (exact exp, fused row accumulate) and
VectorE (Schraudolph float-bits int16 trick + 4x bf16 reduce), ratio
tunable via n_sc.
"""

import math

import numpy as np

import concourse.bass as bass
import concourse.bacc as bacc
import concourse.tile as tile
from concourse import mybir

N = 8192
D = 128
NCLASS = 100
NCORES = 8
CH = 64  # 128-row chunks
BAND = 33  # chunks per strip (diag + 32)
LCOLS = 7 * 128 + BAND * 128  # local column span = 5120
CSPAN_LO, CSPAN_HI = 128, 7 * 128 + 4096  # useful colsum range
TEMP_INV = 10.0

LOG2E = math.log2(math.e)
C16 = TEMP_INV * LOG2E * 2.0**16  # embedding scale^2 folded in on host
ACT_SCALE = math.log(2.0) / 2.0**16  # psum -> exp argument (with bias -10)
SCH_SIGMA = 0.0566  # Schraudolph mean-bias correction
SCH_B16 = (127.0 - SCH_SIGMA - TEMP_INV * LOG2E) * 2.0**7
SCH_K16 = 2.0**-9

F32 = mybir.dt.float32
BF16 = mybir.dt.bfloat16
I16 = mybir.dt.int16

SGW = 1024  # colacc super-group width (2 PSUM banks, double-buffered)
GROUPS = [(i * SGW, (i + 1) * SGW) for i in range(LCOLS // SGW)]
PIECE_W = 1024  # max exp piece width (psum tile, 2 banks)
COPY_ENG = ["S", "D", "S", "D", "S"]  # colacc->SBUF copy engine per group


def schedule(n_sc: int = 29):
    """Static per-core schedule (identical on all cores).

    Returns (groups, nslot): groups is a list of
    dict(G0, G1, batches=[dict(k, pieces=[(p0, p1, eng, slot)],
    cols=[(a, b, start, stop)])]).
    """
    slot = 0
    groups = []
    pieces_flat = []
    for G0, G1 in GROUPS:
        batches = []
        for k in range(8):
            cs, ce = max(k * 128, G0), min(k * 128 + BAND * 128, G1)
            if cs >= ce:
                continue
            pieces = []
            p = cs
            # tiny first piece so the exp pipeline starts right after the
            # first small DMA slice lands
            if cs == 0:
                pieces.append([0, 256, None, slot])
                slot += 1
                p = 256
            while p < ce:
                pe_ = min(p + PIECE_W, ce)
                pieces.append([p, pe_, None, slot])
                slot += 1
                p = pe_
            pieces_flat.extend(pieces)
            # colsum range: strip cols minus diag chunk minus antipode chunk
            c0, c1 = max(k * 128 + 128, cs), min(k * 128 + 4096, ce)
            cols = []
            if c0 < c1:
                bounds = {c0, c1}
                bounds.update(b for b in range(0, LCOLS + 1, 512) if c0 < b < c1)
                bounds.update(
                    pp for pc in pieces for pp in pc[:2] if c0 < pp < c1
                )
                bs = sorted(bounds)
                cols = [[a, b, False, False] for a, b in zip(bs[:-1], bs[1:])]
            batches.append(dict(k=k, cs=cs, ce=ce, pieces=pieces, cols=cols))
        groups.append(dict(G0=G0, G1=G1, batches=batches))

    # Engine assignment.  Diag chunks may land on either engine: the host
    # subtracts an engine-matched diag term (exact exp for ScalarE pieces,
    # a Schraudolph replica for VectorE pieces).
    if n_sc < 0:
        # width-aware greedy: assign each piece (in emission order) to the
        # engine that finishes it sooner, using per-instruction cost models
        # (ScalarE activation vs VectorE convert+reduce).  Copies pre-load
        # both engines.
        load_s = 5 * (172 + 512) / 1.2  # 5 colacc half-copies on ScalarE
        load_d = 5 * (120 + 512) / 0.96  # 5 on VectorE
        for pc in pieces_flat:
            w = pc[1] - pc[0]
            cs_ = (172 + w) / 1.2 + 70
            cd_ = (178 + 2 * w) / 0.96 + 100
            if load_s + cs_ <= load_d + cd_:
                pc[2] = "S"
                load_s += cs_
            else:
                pc[2] = "D"
                load_d += cd_
    else:
        total = len(pieces_flat)
        s_used = 0
        for i, pc in enumerate(pieces_flat):
            if s_used * total < n_sc * (i + 1) and s_used < n_sc:
                pc[2] = "S"
                s_used += 1
            else:
                pc[2] = "D"

    # start/stop flags per (group, psum bank of colacc)
    for g in groups:
        first_seen, last_mm = {}, {}
        for batch in g["batches"]:
            for cm in batch["cols"]:
                bank = (cm[0] - g["G0"]) // 512
                if bank not in first_seen:
                    cm[2] = True
                    first_seen[bank] = True
                last_mm[bank] = cm
        for cm in last_mm.values():
            cm[3] = True
    return groups, slot


def build_nc(
    loop_k: int = 1,
    n_sc: int = 29,
    stage: int = 3,
    preload: int = 0,
    warm: int = 4,
):
    groups, nslot = schedule(n_sc)
    nc = bacc.Bacc("TRN2", target_bir_lowering=False, debug=False)

    xt_d = nc.dram_tensor("xt", [128, LCOLS], BF16, kind="ExternalInput")
    rowp_d = nc.dram_tensor("rowp", [128, nslot], F32, kind="ExternalOutput")
    colp_d = nc.dram_tensor("colp", [1, LCOLS], F32, kind="ExternalOutput")

    with tile.TileContext(nc) as tc:
        with (
            tc.tile_pool(name="persist", bufs=1) as persist,
            tc.tile_pool(name="expool", bufs=6) as expool,
            tc.tile_pool(name="mpsum", bufs=3, space="PSUM") as mpsum,
            tc.tile_pool(name="colpsum", bufs=2, space="PSUM") as colpsum,
        ):
            xt = persist.tile([128, LCOLS], BF16, tag="xt")
            ones = persist.tile([128, 1], BF16, tag="ones")
            warmt = persist.tile([128, 512], BF16, tag="warmt")
            nc.vector.memset(warmt[:], 1.0)
            bneg10 = persist.tile([128, 1], F32, tag="bneg10")
            rowp = persist.tile([128, nslot], F32, tag="rowp")
            colsb = persist.tile([1, LCOLS], F32, tag="colsb")

            nc.vector.memset(ones[:], 1.0)
            nc.vector.memset(bneg10[:], -TEMP_INV)
            if stage < 2:
                nc.vector.memset(rowp[:], 0.0)

            def load_xt():
                for a, b in [(0, 256), (256, 1024)] + [
                    (i * 1024, (i + 1) * 1024) for i in range(1, 5)
                ]:
                    nc.sync.dma_start(xt[:, a:b], xt_d[:, a:b])

            def body():
                if not preload:
                    load_xt()
                if warm:
                    # dummy matmuls during the initial DMA wait keep the PE
                    # HAM activity window busy so real matmuls start warm
                    wps = colpsum.tile([128, 512], F32, tag="colacc", name="wps")
                    for _ in range(warm):
                        nc.tensor.matmul(
                            wps[:], warmt[:, 0:128], warmt[:], start=True,
                            stop=True, skip_group_check=True,
                        )

                for gi, g in enumerate(groups):
                    G0, G1 = g["G0"], g["G1"]
                    # two single-bank colacc halves (pool bufs=2 -> the next
                    # group's accumulation doesn't wait on this group's copy)
                    colacc_h = [
                        colpsum.tile(
                            [128, 512], F32, tag="colacc", name=f"colacc{h}"
                        )
                        for h in range((G1 - G0) // 512)
                    ]
                    ext_of = {}  # piece p0 -> (tile, p0, bf16_view)

                    def emit_colsums(batch):
                        for a, b, st, sp in batch["cols"]:
                            # find the piece containing [a, b)
                            for p0, p1, eng, slot_ in batch["pieces"]:
                                if p0 <= a and b <= p1:
                                    break
                            ext, q0, view = ext_of[p0]
                            h = (a - G0) // 512
                            nc.tensor.matmul(
                                colacc_h[h][0:1, a - G0 - 512 * h : b - G0 - 512 * h],
                                ones[:],
                                view[:, a - q0 : b - q0],
                                start=st,
                                stop=sp,
                                skip_group_check=True,
                            )

                    pending = None
                    for batch in g["batches"]:
                        k = batch["k"]
                        lhsT = xt[:, k * 128 : (k + 1) * 128]
                        for p0, p1, eng, slot_ in batch["pieces"]:
                            w = p1 - p0
                            ps = mpsum.tile([128, PIECE_W], F32, tag="ps")
                            for a in range(0, w, 512):
                                b = min(a + 512, w)
                                nc.tensor.matmul(
                                    ps[:, a:b],
                                    lhsT,
                                    xt[:, p0 + a : p0 + b],
                                    start=True,
                                    stop=True,
                                )
                            if stage < 2:
                                continue
                            if eng == "S":
                                ext = expool.tile(
                                    [128, PIECE_W], BF16, tag="extS", name="extS"
                                )
                                nc.scalar.activation(
                                    ext[:, :w],
                                    ps[:, :w],
                                    mybir.ActivationFunctionType.Exp,
                                    bias=bneg10[:],
                                    scale=ACT_SCALE,
                                    accum_out=rowp[:, slot_ : slot_ + 1],
                                )
                                ext_of[p0] = (ext, p0, ext)
                            else:
                                ext = expool.tile(
                                    [128, PIECE_W], I16, tag="extD", name="extD"
                                )
                                nc.vector.tensor_scalar(
                                    out=ext[:, :w],
                                    in0=ps[:, :w],
                                    scalar1=SCH_K16,
                                    scalar2=SCH_B16,
                                    op0=mybir.AluOpType.mult,
                                    op1=mybir.AluOpType.add,
                                )
                                nc.vector.tensor_reduce(
                                    out=rowp[:, slot_ : slot_ + 1],
                                    in_=ext[:, :w].bitcast(BF16),
                                    axis=mybir.AxisListType.X,
                                    op=mybir.AluOpType.add,
                                )
                                ext_of[p0] = (ext, p0, ext.bitcast(BF16))
                        if stage >= 3 and pending is not None:
                            emit_colsums(pending)
                        pending = batch
                    if stage >= 3:
                        emit_colsums(pending)
                        if gi == len(groups) - 1:
                            # rowp is complete once the last exps retire;
                            # start its DMA before the final copy+DMA tail
                            nc.sync.dma_start(rowp_d[:], rowp[:])
                        # copy halves on different engines (different banks)
                        nc.scalar.copy(
                            colsb[0:1, G0 : G0 + 512], colacc_h[0][0:1, :]
                        )
                        nc.vector.tensor_copy(
                            colsb[0:1, G0 + 512 : G1], colacc_h[1][0:1, :]
                        )
                        nc.sync.dma_start(colp_d[0:1, G0:G1], colsb[0:1, G0:G1])

                if stage < 3:
                    nc.sync.dma_start(rowp_d[:], rowp[:])

            if preload:
                load_xt()
            if loop_k == 1:
                body()
            else:
                with tc.For_i(0, loop_k, 1):
                    body()

    nc.compile()
    return nc


def prepare_inputs(embeddings: np.ndarray, labels: np.ndarray):
    """Host prep: normalize+scale rows to bf16; per-core rotated transposed
    views (only local columns [0, 5120)).  Returns (in_maps, aux) where aux
    carries everything the host combine step needs."""
    import ml_dtypes

    x = np.asarray(embeddings, dtype=np.float64)
    rn = 1.0 / np.maximum(np.sqrt((x * x).sum(axis=1)), 1e-12)
    e = x * rn[:, None]
    xn = (e * math.sqrt(C16)).astype(ml_dtypes.bfloat16)

    in_maps = []
    for c in range(NCORES):
        loc = np.roll(xn, -c * 1024, axis=0)
        xt_host = np.ascontiguousarray(loc.T[:, :LCOLS])
        in_maps.append({"xt": xt_host})

    lab = np.asarray(labels).astype(np.int64).ravel()
    xnf = xn.astype(np.float64)
    gdiag = (xnf * xnf).sum(axis=1) / C16
    diag_exact = np.exp(TEMP_INV * gdiag - TEMP_INV)
    # Schraudolph replica of the device's VectorE path for the diag term
    z = np.rint(gdiag * C16 * SCH_K16 + SCH_B16).astype(np.int16)
    diag_sch = z.view(np.uint16).astype(np.uint16).view(ml_dtypes.bfloat16)
    diag_sch = diag_sch.astype(np.float64)
    counts = np.bincount(lab, minlength=NCLASS).astype(np.float64)
    S = np.zeros((NCLASS, D))
    np.add.at(S, lab, e)
    d = np.einsum("ij,ij->i", e, S[lab])
    Ci = counts[lab] - 1.0
    aux = dict(diag_exact=diag_exact, diag_sch=diag_sch, d=d, Ci=Ci)
    return in_maps, aux


def combine(results, aux, n_sc: int = 29) -> np.ndarray:
    """Host combine: per-core rowp/colp partials -> loss."""
    groups, nslot = schedule(n_sc)
    # strip of each slot
    slot_strip = np.empty(nslot, dtype=np.int64)
    for g in groups:
        for batch in g["batches"]:
            for p0, p1, eng, slot_ in batch["pieces"]:
                slot_strip[slot_] = batch["k"]

    rowsum = np.zeros(N)
    colsum = np.zeros(N)
    ridx = np.arange(1024)
    cidx = np.arange(CSPAN_LO, CSPAN_HI)
    for c in range(NCORES):
        rowp = np.asarray(results[c]["rowp"], dtype=np.float64)  # [128, nslot]
        colp = np.asarray(results[c]["colp"], dtype=np.float64).ravel()
        lrow = np.zeros(1024)
        for k in range(8):
            sl = np.where(slot_strip == k)[0]
            lrow[k * 128 : (k + 1) * 128] = rowp[:, sl].sum(axis=1)
        rowsum[(ridx + c * 1024) % N] += lrow
        colsum[(cidx + c * 1024) % N] += colp[CSPAN_LO:CSPAN_HI]

    # per-row diag term: rows of local strip k use the engine of the piece
    # holding column k*128 in group 0 (same schedule on every core)
    diag_eng = {}
    for batch in groups[0]["batches"]:
        for p0, p1, eng, _s in batch["pieces"]:
            if p0 <= batch["k"] * 128 < p1:
                diag_eng[batch["k"]] = eng
    diag_term = aux["diag_exact"].copy()
    for c in range(NCORES):
        for k in range(8):
            if diag_eng[k] == "D":
                r0 = c * 1024 + k * 128
                diag_term[r0 : r0 + 128] = aux["diag_sch"][r0 : r0 + 128]

    sumexp = rowsum + colsum - diag_term
    logz = TEMP_INV + np.log(sumexp)
    row_loss = aux["Ci"] * logz - TEMP_INV * (aux["d"] - 1.0)
    return np.asarray(row_loss.sum() / N, dtype=np.float32)


_NC_CACHE = {}


def kernel(embeddings: np.ndarray, labels: np.ndarray) -> np.ndarray:
    from concourse.bass_utils import run_bass_kernel_spmd

    nc = _NC_CACHE.get("nc")
    if nc is None:
        nc = _NC_CACHE["nc"] = build_nc(loop_k=1)
    in_maps, aux = prepare_inputs(embeddings, labels)
    res = run_bass_kernel_spmd(nc, in_maps, list(range(NCORES)))
    return combine(res.results, aux)
